# revision 1
# baseline (speedup 1.0000x reference)
"""Trainium2 Bass kernel for nn_DigitLayer (CapsNet digit-capsule layer).

Math note: the reference's routing softmax acts on a size-1 axis, so the
coupling coefficients are exactly 1.0 on every iteration and the whole
3-iteration routing loop collapses to

    S[b,d,i] = sum_{p,j} W[p,d,i,j] * x[b,p,j]
    out      = squash(S)  over i (the 16-dim)

i.e. one [B, P*8] @ [P*8, D*16] matmul plus a per-(b,d) squash.

Distribution: the contraction dim P (1152) is sharded across the 8 cores so
every byte of x and W is read from HBM exactly once chip-wide (pure data
parallel would re-read the 5.9MB W on every core). Each core computes a
partial S over its P-shard for all 256 batches; the host sums the 8 partials
and applies the squash.
"""

import numpy as np

import concourse.bacc as bacc
import concourse.tile as tile
import concourse.mybir as mybir
from concourse.bass_utils import run_bass_kernel_spmd

B, P, D, VP, VD = 256, 1152, 10, 8, 16
NCORES = 8
PL = P // NCORES           # 144 primary capsules per core
KL = PL * VP               # 1152 local contraction length
KCH = KL // 128            # 9 chunks of 128
N_OUT = D * VD             # 160
MB = 128                   # batch chunk (matmul M / PSUM partitions)
NMB = B // MB              # 2

_cache = {}


def _build(dt_mode="f32"):
    dt = mybir.dt.float32
    nc = bacc.Bacc("TRN2", debug=False, num_devices=NCORES)
    xt = nc.dram_tensor("xt", [128, KCH, B], dt, kind="ExternalInput").ap()
    wt = nc.dram_tensor("wt", [128, KCH, N_OUT], dt, kind="ExternalInput").ap()
    out = nc.dram_tensor("out", [B, N_OUT], mybir.dt.float32, kind="ExternalOutput").ap()

    with tile.TileContext(nc) as tc:
        with (
            tc.tile_pool(name="xp", bufs=1) as xp,
            tc.tile_pool(name="wp", bufs=1) as wp,
            tc.tile_pool(name="op", bufs=2) as op,
            tc.tile_pool(name="ps", bufs=2, space="PSUM") as ps,
        ):
            xsb = xp.tile([128, KCH, B], dt)
            wsb = wp.tile([128, KCH, N_OUT], dt)
            for k in range(KCH):
                nc.sync.dma_start(out=wsb[:, k, :], in_=wt[:, k, :])
                nc.sync.dma_start(out=xsb[:, k, :], in_=xt[:, k, :])
            for m in range(NMB):
                pt = ps.tile([MB, N_OUT], mybir.dt.float32)
                for k in range(KCH):
                    nc.tensor.matmul(
                        pt[:],
                        lhsT=xsb[:, k, m * MB:(m + 1) * MB],
                        rhs=wsb[:, k, :],
                        start=(k == 0),
                        stop=(k == KCH - 1),
                    )
                ot = op.tile([MB, N_OUT], mybir.dt.float32)
                nc.vector.tensor_copy(ot[:], pt[:])
                nc.sync.dma_start(out=out[m * MB:(m + 1) * MB, :], in_=ot[:])
    nc.compile()
    return nc


def _prep_inputs(x, W):
    """Per-core host-side layout: SBUF-native [128, KCH, *] arrays."""
    xs = np.ascontiguousarray(x[..., 0], dtype=np.float32)      # [B, P, 8]
    in_maps = []
    for c in range(NCORES):
        pr = slice(c * PL, (c + 1) * PL)
        # x^T chunk: [128, KCH, B] with k_local = kch*128 + kp = p_local*8 + j
        xl = xs[:, pr, :].reshape(B, KL).T                      # [KL, B]
        xl = xl.reshape(KCH, 128, B).transpose(1, 0, 2)         # [128, KCH, B]
        # W2 chunk: W2[(p_local, j), (d, i)] = W[p, d, i, j]
        wl = W[pr].transpose(0, 3, 1, 2).reshape(KL, N_OUT)     # [KL, 160]
        wl = wl.reshape(KCH, 128, N_OUT).transpose(1, 0, 2)     # [128, KCH, 160]
        in_maps.append({
            "xt": np.ascontiguousarray(xl),
            "wt": np.ascontiguousarray(wl),
        })
    return in_maps


def _squash(S):
    """S: [B, 160] full sum -> squash over each group of 16."""
    S = S.reshape(B, D, VD)
    sq = np.sum(S * S, axis=2, keepdims=True)
    v = S * sq / (1.0 + sq) / np.sqrt(sq + 1e-9)
    return v[..., None].astype(np.float32)                      # [B, D, 16, 1]


def run(x, W, trace=False):
    if "nc" not in _cache:
        _cache["nc"] = _build()
    nc = _cache["nc"]
    in_maps = _prep_inputs(x, W)
    res = run_bass_kernel_spmd(nc, in_maps, core_ids=list(range(NCORES)), trace=trace)
    S = np.zeros((B, N_OUT), dtype=np.float32)
    for c in range(NCORES):
        S += res.results[c]["out"]
    return _squash(S), res


def kernel(x, W):
    out, _ = run(np.asarray(x), np.asarray(W))
    return out


# revision 32
# speedup vs baseline: 1.6540x; 1.6540x over previous
"""Trainium2 Bass kernel for nn_DigitLayer (CapsNet digit-capsule layer).

Math note: the reference's routing softmax acts on a size-1 axis, so the
coupling coefficients are exactly 1.0 on every iteration and the whole
3-iteration routing loop collapses to

    S[b,d,i] = sum_{p,j} W[p,d,i,j] * x[b,p,j]
    out      = squash(S)  over i (the 16-dim)

i.e. one [B, P*8] @ [P*8, D*16] matmul plus a per-(b,d) squash.

Distribution: the contraction dim P (1152) is sharded across the 8 cores so
every byte of x and W is read from HBM exactly once chip-wide (pure data
parallel would re-read the replicated 5.9MB W on every core: ~7.1MB/core of
HBM traffic vs ~2.0MB/core here). Each core computes a partial
S[b, (d,i)] = sum_k xT[k,b] * W2[k,(d,i)] over its P-shard for all 256
batches via 18 accumulating PE matmuls; the host sums the 8 partial tensors
and applies the (collapsed-routing) squash.

Inputs are fed to the device as float16: the PE runs f16 at full rate
(1 cycle/row vs 4 for fp32) and the DMA bytes halve; measured end-to-end
relative error is ~4e-4 (f16 keeps 11 mantissa bits and accumulation is
fp32 in PSUM).

Device-side layout (per core, all host-prepped, SBUF-native):
    xt [128, 9, 256] f16 : xT chunks, k_local = kc*128 + kp = p_local*8 + j
    wt [128, 9, 160] f16 : W2 chunks, same k mapping, n = d*16 + i
    out [256, 160] f32   : partial S
"""

import numpy as np

import concourse.bacc as bacc
import concourse.mybir as mybir
from concourse.bass_utils import run_bass_kernel_spmd

B, P, D, VP, VD = 256, 1152, 10, 8, 16
NCORES = 8
PL = P // NCORES           # 144 primary capsules per core
KL = PL * VP               # 1152 local contraction length
KCH = KL // 128            # 9 k-chunks of 128
N_OUT = D * VD             # 160
MB = 128                   # batch chunk (matmul M / PSUM partitions)
NMB = B // MB              # 2
GROUPS = (1, 4, 4)         # k-chunks per input-DMA group

_cache = {}


def _hoist_first(nc, instrs):
    """Move the given instructions to the front of their engine's stream so
    the input DMAs issue before the framework preamble (const memsets +
    all-engine barrier) and their transfer latency overlaps it."""
    names = {i.name for i in instrs}
    for bb in nc.main_func.blocks:
        if not any(ins.name in names for ins in bb.instructions):
            continue
        by_engine = {}
        for ins in bb.instructions:
            if ins.name in names:
                by_engine.setdefault(ins.engine, []).append(ins)
        new = []
        emitted = set()
        for ins in bb.instructions:
            if ins.name in names:
                continue
            e = ins.engine
            if e in by_engine and e not in emitted:
                new.extend(by_engine[e])
                emitted.add(e)
            new.append(ins)
        for e, lst in by_engine.items():
            if e not in emitted:
                new.extend(lst)
        bb.instructions[:] = new


def _build():
    """Raw-bass kernel (no TileContext), hand-placed semaphores.

    Hard-won rules baked in here:
      * One semaphore per DMA: a HWDGE DMA completes as 16 unordered +1
        sub-increments, so intermediate thresholds on a shared sem race.
      * The PE gate must wait on the DMA completion semaphores; an engine
        DRAIN does NOT barrier HWDGE DMA data (cold-run NaNs).
      * PE drain before the DVE reads PSUM: the stop-matmul retire does not
        guarantee the PSUM writes drained, and DVE reading a bank the PE is
        still writing is fatal on HW. Same for DVE drain before the store.
      * No wait on the output DMA semaphore: the walrus end-of-program
        barrier covers it (verified bitwise against the waiting variant).
    """
    dt_in = mybir.dt.float16
    NG = len(GROUPS)
    starts = [sum(GROUPS[:i]) for i in range(NG)]
    nc = bacc.Bacc("TRN2", debug=False, num_devices=NCORES)
    xt = nc.dram_tensor("xt", [128, KCH, B], dt_in, kind="ExternalInput").ap()
    wt = nc.dram_tensor("wt", [128, KCH, N_OUT], dt_in, kind="ExternalInput").ap()
    out = nc.dram_tensor("out", [B, N_OUT], mybir.dt.float32, kind="ExternalOutput").ap()

    from contextlib import ExitStack
    with ExitStack() as ctx:
        xsb = ctx.enter_context(nc.sbuf_tensor([128, KCH, B], dt_in))
        wsb = ctx.enter_context(nc.sbuf_tensor([128, KCH, N_OUT], dt_in))
        osb = ctx.enter_context(nc.sbuf_tensor([MB, NMB, N_OUT], mybir.dt.float32))
        pts = [
            ctx.enter_context(nc.psum_tensor(f"pt{m}", [MB, N_OUT], mybir.dt.float32))
            for m in range(NMB)
        ]
        sem_xs = [ctx.enter_context(nc.semaphore(name=f"sem_x{g}")) for g in range(NG)]
        sem_ws = [ctx.enter_context(nc.semaphore(name=f"sem_w{g}")) for g in range(NG)]
        sem_mm = ctx.enter_context(nc.semaphore(name="sem_mm"))
        sem_cp = ctx.enter_context(nc.semaphore(name="sem_cp"))
        sem_out = ctx.enter_context(nc.semaphore(name="sem_out"))

        # input DMAs: x on the SP HWDGE ring, w on the ACT ring (parallel)
        in_dmas = []
        for g in range(NG):
            k0, kn = starts[g], GROUPS[g]
            in_dmas.append(nc.sync.dma_start(
                out=xsb[:, k0:k0 + kn, :], in_=xt[:, k0:k0 + kn, :]
            ).then_inc(sem_xs[g], 16).ins)
        for g in range(NG):
            k0, kn = starts[g], GROUPS[g]
            in_dmas.append(nc.scalar.dma_start(
                out=wsb[:, k0:k0 + kn, :], in_=wt[:, k0:k0 + kn, :]
            ).then_inc(sem_ws[g], 16).ins)

        # PE: 18 accumulating matmuls, m outer (one PSUM bank per batch
        # half), group-gated on the input DMAs only on the first pass.
        for m in range(NMB):
            for g in range(NG):
                k0, kn = starts[g], GROUPS[g]
                if m == 0:
                    nc.tensor.wait_ge(sem_xs[g], 16)
                    nc.tensor.wait_ge(sem_ws[g], 16)
                for kk in range(kn):
                    k = k0 + kk
                    nc.tensor.matmul(
                        pts[m][:],
                        lhsT=xsb[:, k, m * MB:(m + 1) * MB],
                        rhs=wsb[:, k, :],
                        start=(k == 0),
                        stop=(k == KCH - 1),
                    )
            nc.tensor.drain().then_inc(sem_mm, 1)
        # DVE: copy each half as soon as its accumulation closes
        for m in range(NMB):
            nc.vector.wait_ge(sem_mm, m + 1)
            nc.vector.tensor_copy(osb[:, m, :], pts[m][:])
        nc.vector.drain().then_inc(sem_cp, 1)
        # SP: single store covering both batch halves
        nc.sync.wait_ge(sem_cp, 1)
        out_r = out.rearrange("(m p) n -> p m n", m=NMB)
        nc.sync.dma_start(out=out_r, in_=osb[:]).then_inc(sem_out, 16)

        _hoist_first(nc, in_dmas)
    nc.compile()
    return nc


def _prep_inputs(x, W):
    """Per-core host-side layout: SBUF-native [128, KCH, *] f16 arrays."""
    xs = np.ascontiguousarray(x[..., 0], dtype=np.float32)      # [B, P, 8]
    W = np.asarray(W, dtype=np.float32)
    in_maps = []
    for c in range(NCORES):
        pr = slice(c * PL, (c + 1) * PL)
        # x^T chunk: [128, KCH, B] with k_local = kc*128 + kp = p_local*8 + j
        xl = xs[:, pr, :].reshape(B, KL).T                      # [KL, B]
        xl = xl.reshape(KCH, 128, B).transpose(1, 0, 2)         # [128, KCH, B]
        # W2 chunk: W2[(p_local, j), (d, i)] = W[p, d, i, j]
        wl = W[pr].transpose(0, 3, 1, 2).reshape(KL, N_OUT)     # [KL, 160]
        wl = wl.reshape(KCH, 128, N_OUT).transpose(1, 0, 2)     # [128, KCH, 160]
        in_maps.append({
            "xt": np.ascontiguousarray(xl, dtype=np.float16),
            "wt": np.ascontiguousarray(wl, dtype=np.float16),
        })
    return in_maps


def _squash(S):
    """S: [B, 160] summed partials -> squash over each group of 16."""
    S = S.reshape(B, D, VD)
    sq = np.sum(S * S, axis=2, keepdims=True)
    v = S * sq / (1.0 + sq) / np.sqrt(sq + 1e-9)
    return v[..., None].astype(np.float32)                      # [B, D, 16, 1]


def run(x, W, trace=False):
    if "nc" not in _cache:
        _cache["nc"] = _build()
    nc = _cache["nc"]
    in_maps = _prep_inputs(x, W)
    try:
        res = run_bass_kernel_spmd(nc, in_maps, core_ids=list(range(NCORES)), trace=trace)
    except Exception:
        # one retry absorbs transient runtime hiccups
        res = run_bass_kernel_spmd(nc, in_maps, core_ids=list(range(NCORES)), trace=trace)
    S = np.zeros((B, N_OUT), dtype=np.float32)
    for c in range(NCORES):
        S += res.results[c]["out"]
    return _squash(S), res


def kernel(x, W):
    out, _ = run(np.asarray(x), np.asarray(W))
    return out


# revision 34
# speedup vs baseline: 1.7115x; 1.0348x over previous
"""Trainium2 Bass kernel for nn_DigitLayer (CapsNet digit-capsule layer).

Math note: the reference's routing softmax acts on a size-1 axis, so the
coupling coefficients are exactly 1.0 on every iteration and the whole
3-iteration routing loop collapses to

    S[b,d,i] = sum_{p,j} W[p,d,i,j] * x[b,p,j]
    out      = squash(S)  over i (the 16-dim)

i.e. one [B, P*8] @ [P*8, D*16] matmul plus a per-(b,d) squash.

Distribution: the contraction dim P (1152) is sharded across the 8 cores so
every byte of x and W is read from HBM exactly once chip-wide (pure data
parallel would re-read the replicated 5.9MB W on every core: ~7.1MB/core of
HBM traffic vs ~2.0MB/core here). Each core computes a partial
S[b, (d,i)] = sum_k xT[k,b] * W2[k,(d,i)] over its P-shard for all 256
batches via 18 accumulating PE matmuls; the host sums the 8 partial tensors
and applies the (collapsed-routing) squash.

Inputs are fed to the device as float16: the PE runs f16 at full rate
(1 cycle/row vs 4 for fp32) and the DMA bytes halve; measured end-to-end
relative error is ~4e-4 (f16 keeps 11 mantissa bits and accumulation is
fp32 in PSUM).

Device-side layout (per core, all host-prepped, SBUF-native):
    xt [128, 9, 256] f16 : xT chunks, k_local = kc*128 + kp = p_local*8 + j
    wt [128, 9, 160] f16 : W2 chunks, same k mapping, n = d*16 + i
    out [256, 160] f32   : partial S
"""

import numpy as np

import concourse.bacc as bacc
import concourse.mybir as mybir
from concourse.bass_utils import run_bass_kernel_spmd

B, P, D, VP, VD = 256, 1152, 10, 8, 16
NCORES = 8
PL = P // NCORES           # 144 primary capsules per core
KL = PL * VP               # 1152 local contraction length
KCH = KL // 128            # 9 k-chunks of 128
N_OUT = D * VD             # 160
MB = 128                   # batch chunk (matmul M / PSUM partitions)
NMB = B // MB              # 2
GROUPS = (1, 4, 4)         # k-chunks per input-DMA group

_cache = {}


def _hoist_first(nc, instrs):
    """Move the given instructions to the front of their engine's stream so
    the input DMAs issue before the framework preamble (const memsets +
    all-engine barrier) and their transfer latency overlaps it."""
    names = {i.name for i in instrs}
    for bb in nc.main_func.blocks:
        if not any(ins.name in names for ins in bb.instructions):
            continue
        by_engine = {}
        for ins in bb.instructions:
            if ins.name in names:
                by_engine.setdefault(ins.engine, []).append(ins)
        new = []
        emitted = set()
        for ins in bb.instructions:
            if ins.name in names:
                continue
            e = ins.engine
            if e in by_engine and e not in emitted:
                new.extend(by_engine[e])
                emitted.add(e)
            new.append(ins)
        for e, lst in by_engine.items():
            if e not in emitted:
                new.extend(lst)
        bb.instructions[:] = new


def _build():
    """Raw-bass kernel (no TileContext), hand-placed semaphores.

    Hard-won rules baked in here:
      * One semaphore per DMA: a HWDGE DMA completes as 16 unordered +1
        sub-increments, so intermediate thresholds on a shared sem race.
      * The PE gate must wait on the DMA completion semaphores; an engine
        DRAIN does NOT barrier HWDGE DMA data (cold-run NaNs).
      * PE drain before the DVE reads PSUM: the stop-matmul retire does not
        guarantee the PSUM writes drained, and DVE reading a bank the PE is
        still writing is fatal on HW. Same for DVE drain before the store.
      * No wait on the output DMA semaphore: the walrus end-of-program
        barrier covers it (verified bitwise against the waiting variant).
    """
    dt_in = mybir.dt.float16
    NG = len(GROUPS)
    starts = [sum(GROUPS[:i]) for i in range(NG)]
    nc = bacc.Bacc("TRN2", debug=False, num_devices=NCORES)
    xt = nc.dram_tensor("xt", [128, KCH, B], dt_in, kind="ExternalInput").ap()
    wt = nc.dram_tensor("wt", [128, KCH, N_OUT], dt_in, kind="ExternalInput").ap()
    out = nc.dram_tensor("out", [B, N_OUT], mybir.dt.float32, kind="ExternalOutput").ap()

    from contextlib import ExitStack
    with ExitStack() as ctx:
        xsb = ctx.enter_context(nc.sbuf_tensor([128, KCH, B], dt_in))
        wsb = ctx.enter_context(nc.sbuf_tensor([128, KCH, N_OUT], dt_in))
        osb = ctx.enter_context(nc.sbuf_tensor([MB, NMB, N_OUT], mybir.dt.float32))
        pts = [
            ctx.enter_context(nc.psum_tensor(f"pt{m}", [MB, N_OUT], mybir.dt.float32))
            for m in range(NMB)
        ]
        sem_xs = [ctx.enter_context(nc.semaphore(name=f"sem_x{g}")) for g in range(NG)]
        sem_ws = [ctx.enter_context(nc.semaphore(name=f"sem_w{g}")) for g in range(NG)]
        sem_mm = ctx.enter_context(nc.semaphore(name="sem_mm"))
        sem_cp = ctx.enter_context(nc.semaphore(name="sem_cp"))
        sem_out = ctx.enter_context(nc.semaphore(name="sem_out"))

        # input DMAs: x on the SP HWDGE ring, w on the ACT ring (parallel)
        in_dmas = []
        for g in range(NG):
            k0, kn = starts[g], GROUPS[g]
            in_dmas.append(nc.sync.dma_start(
                out=xsb[:, k0:k0 + kn, :], in_=xt[:, k0:k0 + kn, :]
            ).then_inc(sem_xs[g], 16).ins)
        for g in range(NG):
            k0, kn = starts[g], GROUPS[g]
            in_dmas.append(nc.scalar.dma_start(
                out=wsb[:, k0:k0 + kn, :], in_=wt[:, k0:k0 + kn, :]
            ).then_inc(sem_ws[g], 16).ins)

        # PE: 18 accumulating matmuls, m INNER (the two batch halves
        # accumulate into separate PSUM banks concurrently), group-gated on
        # the input DMAs — so after the last DMA gate only the last group's
        # matmuls remain, not a whole batch half. In the last group, half 0
        # closes first so its copy/store overlap half 1's matmuls (the copy
        # reads a different PSUM bank than the PE is writing).
        for g in range(NG):
            k0, kn = starts[g], GROUPS[g]
            nc.tensor.wait_ge(sem_xs[g], 16)
            nc.tensor.wait_ge(sem_ws[g], 16)
            if g < NG - 1:
                for kk in range(kn):
                    k = k0 + kk
                    for m in range(NMB):
                        nc.tensor.matmul(
                            pts[m][:],
                            lhsT=xsb[:, k, m * MB:(m + 1) * MB],
                            rhs=wsb[:, k, :],
                            start=(k == 0),
                            stop=(k == KCH - 1),
                        )
            else:
                for m in range(NMB):
                    for kk in range(kn):
                        k = k0 + kk
                        mm = nc.tensor.matmul(
                            pts[m][:],
                            lhsT=xsb[:, k, m * MB:(m + 1) * MB],
                            rhs=wsb[:, k, :],
                            start=(k == 0),
                            stop=(k == KCH - 1),
                        )
                    if m == 0:
                        mm.then_inc(sem_mm, 1)
                    else:
                        nc.tensor.drain().then_inc(sem_mm, 1)
        # DVE: copy each half as soon as its accumulation closes; per-copy
        # drain so each store reads settled SBUF.
        for m in range(NMB):
            nc.vector.wait_ge(sem_mm, m + 1)
            nc.vector.tensor_copy(osb[:, m, :], pts[m][:])
            nc.vector.drain().then_inc(sem_cp, 1)
        # SP: per-half stores
        for m in range(NMB):
            nc.sync.wait_ge(sem_cp, m + 1)
            nc.sync.dma_start(
                out=out[m * MB:(m + 1) * MB, :], in_=osb[:, m, :]
            ).then_inc(sem_out, 16)

        _hoist_first(nc, in_dmas)
    nc.compile()
    return nc


def _prep_inputs(x, W):
    """Per-core host-side layout: SBUF-native [128, KCH, *] f16 arrays."""
    xs = np.ascontiguousarray(x[..., 0], dtype=np.float32)      # [B, P, 8]
    W = np.asarray(W, dtype=np.float32)
    in_maps = []
    for c in range(NCORES):
        pr = slice(c * PL, (c + 1) * PL)
        # x^T chunk: [128, KCH, B] with k_local = kc*128 + kp = p_local*8 + j
        xl = xs[:, pr, :].reshape(B, KL).T                      # [KL, B]
        xl = xl.reshape(KCH, 128, B).transpose(1, 0, 2)         # [128, KCH, B]
        # W2 chunk: W2[(p_local, j), (d, i)] = W[p, d, i, j]
        wl = W[pr].transpose(0, 3, 1, 2).reshape(KL, N_OUT)     # [KL, 160]
        wl = wl.reshape(KCH, 128, N_OUT).transpose(1, 0, 2)     # [128, KCH, 160]
        in_maps.append({
            "xt": np.ascontiguousarray(xl, dtype=np.float16),
            "wt": np.ascontiguousarray(wl, dtype=np.float16),
        })
    return in_maps


def _squash(S):
    """S: [B, 160] summed partials -> squash over each group of 16."""
    S = S.reshape(B, D, VD)
    sq = np.sum(S * S, axis=2, keepdims=True)
    v = S * sq / (1.0 + sq) / np.sqrt(sq + 1e-9)
    return v[..., None].astype(np.float32)                      # [B, D, 16, 1]


def run(x, W, trace=False):
    if "nc" not in _cache:
        _cache["nc"] = _build()
    nc = _cache["nc"]
    in_maps = _prep_inputs(x, W)
    try:
        res = run_bass_kernel_spmd(nc, in_maps, core_ids=list(range(NCORES)), trace=trace)
    except Exception:
        # one retry absorbs transient runtime hiccups
        res = run_bass_kernel_spmd(nc, in_maps, core_ids=list(range(NCORES)), trace=trace)
    S = np.zeros((B, N_OUT), dtype=np.float32)
    for c in range(NCORES):
        S += res.results[c]["out"]
    return _squash(S), res


def kernel(x, W):
    out, _ = run(np.asarray(x), np.asarray(W))
    return out


# revision 36
# speedup vs baseline: 1.7229x; 1.0066x over previous
"""Trainium2 Bass kernel for nn_DigitLayer (CapsNet digit-capsule layer).

Math note: the reference's routing softmax acts on a size-1 axis, so the
coupling coefficients are exactly 1.0 on every iteration and the whole
3-iteration routing loop collapses to

    S[b,d,i] = sum_{p,j} W[p,d,i,j] * x[b,p,j]
    out      = squash(S)  over i (the 16-dim)

i.e. one [B, P*8] @ [P*8, D*16] matmul plus a per-(b,d) squash.

Distribution: the contraction dim P (1152) is sharded across the 8 cores so
every byte of x and W is read from HBM exactly once chip-wide (pure data
parallel would re-read the replicated 5.9MB W on every core: ~7.1MB/core of
HBM traffic vs ~2.0MB/core here). Each core computes a partial
S[b, (d,i)] = sum_k xT[k,b] * W2[k,(d,i)] over its P-shard for all 256
batches via 18 accumulating PE matmuls; the host sums the 8 partial tensors
and applies the (collapsed-routing) squash.

Inputs are fed to the device as float16: the PE runs f16 at full rate
(1 cycle/row vs 4 for fp32) and the DMA bytes halve; measured end-to-end
relative error is ~4e-4 (f16 keeps 11 mantissa bits and accumulation is
fp32 in PSUM).

Device-side layout (per core, all host-prepped, SBUF-native):
    xt [128, 9, 256] f16 : xT chunks, k_local = kc*128 + kp = p_local*8 + j
    wt [128, 9, 160] f16 : W2 chunks, same k mapping, n = d*16 + i
    out [256, 160] f32   : partial S
"""

import numpy as np

import concourse.bacc as bacc
import concourse.mybir as mybir
from concourse.bass_utils import run_bass_kernel_spmd

B, P, D, VP, VD = 256, 1152, 10, 8, 16
NCORES = 8
PL = P // NCORES           # 144 primary capsules per core
KL = PL * VP               # 1152 local contraction length
KCH = KL // 128            # 9 k-chunks of 128
N_OUT = D * VD             # 160
MB = 128                   # batch chunk (matmul M / PSUM partitions)
NMB = B // MB              # 2
GROUPS = (1, 4, 4)         # k-chunks per input-DMA group

_cache = {}


def _hoist_first(nc, instrs):
    """Move the given instructions to the front of their engine's stream so
    the input DMAs issue before the framework preamble (const memsets +
    all-engine barrier) and their transfer latency overlaps it."""
    names = {i.name for i in instrs}
    for bb in nc.main_func.blocks:
        if not any(ins.name in names for ins in bb.instructions):
            continue
        by_engine = {}
        for ins in bb.instructions:
            if ins.name in names:
                by_engine.setdefault(ins.engine, []).append(ins)
        new = []
        emitted = set()
        for ins in bb.instructions:
            if ins.name in names:
                continue
            e = ins.engine
            if e in by_engine and e not in emitted:
                new.extend(by_engine[e])
                emitted.add(e)
            new.append(ins)
        for e, lst in by_engine.items():
            if e not in emitted:
                new.extend(lst)
        bb.instructions[:] = new


def _build():
    """Raw-bass kernel (no TileContext), hand-placed semaphores.

    Hard-won rules baked in here:
      * One semaphore per DMA: a HWDGE DMA completes as 16 unordered +1
        sub-increments, so intermediate thresholds on a shared sem race.
      * The PE gate must wait on the DMA completion semaphores; an engine
        DRAIN does NOT barrier HWDGE DMA data (cold-run NaNs).
      * PE drain before the DVE reads PSUM: the stop-matmul retire does not
        guarantee the PSUM writes drained, and DVE reading a bank the PE is
        still writing is fatal on HW. Same for DVE drain before the store.
      * No wait on the output DMA semaphore: the walrus end-of-program
        barrier covers it (verified bitwise against the waiting variant).
    """
    dt_in = mybir.dt.float16
    NG = len(GROUPS)
    starts = [sum(GROUPS[:i]) for i in range(NG)]
    nc = bacc.Bacc("TRN2", debug=False, num_devices=NCORES)
    xt = nc.dram_tensor("xt", [128, KCH, B], dt_in, kind="ExternalInput").ap()
    wt = nc.dram_tensor("wt", [128, KCH, N_OUT], dt_in, kind="ExternalInput").ap()
    out = nc.dram_tensor("out", [B, N_OUT], mybir.dt.float32, kind="ExternalOutput").ap()

    from contextlib import ExitStack
    with ExitStack() as ctx:
        xsb = ctx.enter_context(nc.sbuf_tensor([128, KCH, B], dt_in))
        wsb = ctx.enter_context(nc.sbuf_tensor([128, KCH, N_OUT], dt_in))
        osb = ctx.enter_context(nc.sbuf_tensor([MB, NMB, N_OUT], mybir.dt.float32))
        pts = [
            ctx.enter_context(nc.psum_tensor(f"pt{m}", [MB, N_OUT], mybir.dt.float32))
            for m in range(NMB)
        ]
        sem_xs = [ctx.enter_context(nc.semaphore(name=f"sem_x{g}")) for g in range(NG)]
        sem_ws = [ctx.enter_context(nc.semaphore(name=f"sem_w{g}")) for g in range(NG)]
        sem_mm = ctx.enter_context(nc.semaphore(name="sem_mm"))
        sem_cp = ctx.enter_context(nc.semaphore(name="sem_cp"))
        sem_out = ctx.enter_context(nc.semaphore(name="sem_out"))

        # input DMAs: x on the SP HWDGE ring, w on the ACT ring (parallel)
        in_dmas = []
        for g in range(NG):
            k0, kn = starts[g], GROUPS[g]
            in_dmas.append(nc.sync.dma_start(
                out=xsb[:, k0:k0 + kn, :], in_=xt[:, k0:k0 + kn, :]
            ).then_inc(sem_xs[g], 16).ins)
        for g in range(NG):
            k0, kn = starts[g], GROUPS[g]
            in_dmas.append(nc.scalar.dma_start(
                out=wsb[:, k0:k0 + kn, :], in_=wt[:, k0:k0 + kn, :]
            ).then_inc(sem_ws[g], 16).ins)

        # PE: 18 accumulating matmuls, m INNER (the two batch halves
        # accumulate into separate PSUM banks concurrently), group-gated on
        # the input DMAs — so after the last DMA gate only the last group's
        # matmuls remain, not a whole batch half. In the last group, half 0
        # closes first so its copy/store overlap half 1's matmuls (the copy
        # reads a different PSUM bank than the PE is writing).
        for g in range(NG):
            k0, kn = starts[g], GROUPS[g]
            nc.tensor.wait_ge(sem_xs[g], 16)
            nc.tensor.wait_ge(sem_ws[g], 16)
            if g < NG - 1:
                for kk in range(kn):
                    k = k0 + kk
                    for m in range(NMB):
                        nc.tensor.matmul(
                            pts[m][:],
                            lhsT=xsb[:, k, m * MB:(m + 1) * MB],
                            rhs=wsb[:, k, :],
                            start=(k == 0),
                            stop=(k == KCH - 1),
                        )
            else:
                for m in range(NMB):
                    for kk in range(kn):
                        k = k0 + kk
                        mm = nc.tensor.matmul(
                            pts[m][:],
                            lhsT=xsb[:, k, m * MB:(m + 1) * MB],
                            rhs=wsb[:, k, :],
                            start=(k == 0),
                            stop=(k == KCH - 1),
                        )
                    if m == 0:
                        mm.then_inc(sem_mm, 1)
                    else:
                        nc.tensor.drain().then_inc(sem_mm, 1)
        # DVE: copy each half as soon as its accumulation closes; per-copy
        # drain so each store reads settled SBUF.
        for m in range(NMB):
            nc.vector.wait_ge(sem_mm, m + 1)
            nc.vector.tensor_copy(osb[:, m, :], pts[m][:])
            nc.vector.drain().then_inc(sem_cp, 1)
        # SP: per-half stores
        for m in range(NMB):
            nc.sync.wait_ge(sem_cp, m + 1)
            nc.sync.dma_start(
                out=out[m * MB:(m + 1) * MB, :], in_=osb[:, m, :]
            ).then_inc(sem_out, 16)

        _hoist_first(nc, in_dmas)
    nc.compile()
    return nc


def _prep_inputs(x, W):
    """Per-core host-side layout: SBUF-native [128, KCH, *] f16 arrays."""
    xs = np.ascontiguousarray(x[..., 0], dtype=np.float32)      # [B, P, 8]
    W = np.asarray(W, dtype=np.float32)
    in_maps = []
    for c in range(NCORES):
        pr = slice(c * PL, (c + 1) * PL)
        # x^T chunk: [128, KCH, B] with k_local = kc*128 + kp = p_local*8 + j
        xl = xs[:, pr, :].reshape(B, KL).T                      # [KL, B]
        xl = xl.reshape(KCH, 128, B).transpose(1, 0, 2)         # [128, KCH, B]
        # W2 chunk: W2[(p_local, j), (d, i)] = W[p, d, i, j]
        wl = W[pr].transpose(0, 3, 1, 2).reshape(KL, N_OUT)     # [KL, 160]
        wl = wl.reshape(KCH, 128, N_OUT).transpose(1, 0, 2)     # [128, KCH, 160]
        in_maps.append({
            "xt": np.ascontiguousarray(xl, dtype=np.float16),
            "wt": np.ascontiguousarray(wl, dtype=np.float16),
        })
    return in_maps


def _squash(S):
    """S: [B, 160] summed partials -> squash over each group of 16."""
    S = S.reshape(B, D, VD)
    sq = np.sum(S * S, axis=2, keepdims=True)
    v = S * sq / (1.0 + sq) / np.sqrt(sq + 1e-9)
    return v[..., None].astype(np.float32)                      # [B, D, 16, 1]


def run(x, W, trace=False):
    if "nc" not in _cache:
        _cache["nc"] = _build()
    nc = _cache["nc"]
    in_maps = _prep_inputs(x, W)
    try:
        res = run_bass_kernel_spmd(nc, in_maps, core_ids=list(range(NCORES)), trace=trace)
    except Exception:
        # one retry absorbs transient runtime hiccups
        res = run_bass_kernel_spmd(nc, in_maps, core_ids=list(range(NCORES)), trace=trace)
    S = np.zeros((B, N_OUT), dtype=np.float32)
    for c in range(NCORES):
        S += res.results[c]["out"]
    return _squash(S), res


def kernel(x, W):
    out, _ = run(np.asarray(x), np.asarray(W))
    return out


# revision 38
# speedup vs baseline: 1.7675x; 1.0259x over previous
"""Trainium2 Bass kernel for nn_DigitLayer (CapsNet digit-capsule layer).

Math note: the reference's routing softmax acts on a size-1 axis, so the
coupling coefficients are exactly 1.0 on every iteration and the whole
3-iteration routing loop collapses to

    S[b,d,i] = sum_{p,j} W[p,d,i,j] * x[b,p,j]
    out      = squash(S)  over i (the 16-dim)

i.e. one [B, P*8] @ [P*8, D*16] matmul plus a per-(b,d) squash.

Distribution: the contraction dim P (1152) is sharded across the 8 cores so
every byte of x and W is read from HBM exactly once chip-wide (pure data
parallel would re-read the replicated 5.9MB W on every core: ~7.1MB/core of
HBM traffic vs ~2.0MB/core here). Each core computes a partial
S[b, (d,i)] = sum_k xT[k,b] * W2[k,(d,i)] over its P-shard for all 256
batches via 18 accumulating PE matmuls; the host sums the 8 partial tensors
and applies the (collapsed-routing) squash.

Inputs are fed to the device as float16: the PE runs f16 at full rate
(1 cycle/row vs 4 for fp32) and the DMA bytes halve; measured end-to-end
relative error is ~4e-4 (f16 keeps 11 mantissa bits and accumulation is
fp32 in PSUM).

Device-side layout (per core, all host-prepped, SBUF-native):
    xt [128, 9, 256] f16 : xT chunks, k_local = kc*128 + kp = p_local*8 + j
    wt [128, 9, 160] f16 : W2 chunks, same k mapping, n = d*16 + i
    out [256, 160] f32   : partial S
"""

import numpy as np

import concourse.bacc as bacc
import concourse.mybir as mybir
from concourse.bass_utils import run_bass_kernel_spmd

B, P, D, VP, VD = 256, 1152, 10, 8, 16
NCORES = 8
PL = P // NCORES           # 144 primary capsules per core
KL = PL * VP               # 1152 local contraction length
KCH = KL // 128            # 9 k-chunks of 128
N_OUT = D * VD             # 160
MB = 128                   # batch chunk (matmul M / PSUM partitions)
NMB = B // MB              # 2
GROUPS = (1, 4, 4)         # k-chunks per input-DMA group

_cache = {}


def _hoist_first(nc, instrs):
    """Move the given instructions to the front of their engine's stream so
    the input DMAs issue before the framework preamble (const memsets +
    all-engine barrier) and their transfer latency overlaps it."""
    names = {i.name for i in instrs}
    for bb in nc.main_func.blocks:
        if not any(ins.name in names for ins in bb.instructions):
            continue
        by_engine = {}
        for ins in bb.instructions:
            if ins.name in names:
                by_engine.setdefault(ins.engine, []).append(ins)
        new = []
        emitted = set()
        for ins in bb.instructions:
            if ins.name in names:
                continue
            e = ins.engine
            if e in by_engine and e not in emitted:
                new.extend(by_engine[e])
                emitted.add(e)
            new.append(ins)
        for e, lst in by_engine.items():
            if e not in emitted:
                new.extend(lst)
        bb.instructions[:] = new


def _build():
    """Raw-bass kernel (no TileContext), hand-placed semaphores.

    Hard-won rules baked in here:
      * One semaphore per DMA: a HWDGE DMA completes as 16 unordered +1
        sub-increments, so intermediate thresholds on a shared sem race.
      * The PE gate must wait on the DMA completion semaphores; an engine
        DRAIN does NOT barrier HWDGE DMA data (cold-run NaNs).
      * PE drain before the DVE reads PSUM: the stop-matmul retire does not
        guarantee the PSUM writes drained, and DVE reading a bank the PE is
        still writing is fatal on HW. Same for DVE drain before the store.
      * No wait on the output DMA semaphore: the walrus end-of-program
        barrier covers it (verified bitwise against the waiting variant).
    """
    dt_in = mybir.dt.float16
    NG = len(GROUPS)
    starts = [sum(GROUPS[:i]) for i in range(NG)]
    nc = bacc.Bacc("TRN2", debug=False, num_devices=NCORES)
    xt = nc.dram_tensor("xt", [128, KCH, B], dt_in, kind="ExternalInput").ap()
    wt = nc.dram_tensor("wt", [128, KCH, N_OUT], dt_in, kind="ExternalInput").ap()
    out = nc.dram_tensor("out", [B, N_OUT], mybir.dt.float32, kind="ExternalOutput").ap()

    from contextlib import ExitStack
    with ExitStack() as ctx:
        xsb = ctx.enter_context(nc.sbuf_tensor([128, KCH, B], dt_in))
        wsb = ctx.enter_context(nc.sbuf_tensor([128, KCH, N_OUT], dt_in))
        osb = ctx.enter_context(nc.sbuf_tensor([MB, NMB, N_OUT], mybir.dt.float32))
        pts = [
            ctx.enter_context(nc.psum_tensor(f"pt{m}", [MB, N_OUT], mybir.dt.float32))
            for m in range(NMB)
        ]
        sem_xs = [ctx.enter_context(nc.semaphore(name=f"sem_x{g}")) for g in range(NG)]
        sem_ws = [ctx.enter_context(nc.semaphore(name=f"sem_w{g}")) for g in range(NG)]
        sem_mm = ctx.enter_context(nc.semaphore(name="sem_mm"))
        sem_cp = ctx.enter_context(nc.semaphore(name="sem_cp"))
        sem_out = ctx.enter_context(nc.semaphore(name="sem_out"))

        # input DMAs: x on the SP HWDGE ring, w on the ACT ring (parallel)
        in_dmas = []
        for g in range(NG):
            k0, kn = starts[g], GROUPS[g]
            in_dmas.append(nc.sync.dma_start(
                out=xsb[:, k0:k0 + kn, :], in_=xt[:, k0:k0 + kn, :]
            ).then_inc(sem_xs[g], 16).ins)
        for g in range(NG):
            k0, kn = starts[g], GROUPS[g]
            in_dmas.append(nc.scalar.dma_start(
                out=wsb[:, k0:k0 + kn, :], in_=wt[:, k0:k0 + kn, :]
            ).then_inc(sem_ws[g], 16).ins)

        # PE: 18 accumulating matmuls, m INNER (the two batch halves
        # accumulate into separate PSUM banks concurrently), group-gated on
        # the input DMAs — so after the last DMA gate only the last group's
        # matmuls remain, not a whole batch half. In the last group, half 0
        # closes first so its copy/store overlap half 1's matmuls (the copy
        # reads a different PSUM bank than the PE is writing).
        for g in range(NG):
            k0, kn = starts[g], GROUPS[g]
            nc.tensor.wait_ge(sem_xs[g], 16)
            nc.tensor.wait_ge(sem_ws[g], 16)
            if g < NG - 1:
                for kk in range(kn):
                    k = k0 + kk
                    for m in range(NMB):
                        nc.tensor.matmul(
                            pts[m][:],
                            lhsT=xsb[:, k, m * MB:(m + 1) * MB],
                            rhs=wsb[:, k, :],
                            start=(k == 0),
                            stop=(k == KCH - 1),
                        )
            else:
                for m in range(NMB):
                    for kk in range(kn):
                        k = k0 + kk
                        mm = nc.tensor.matmul(
                            pts[m][:],
                            lhsT=xsb[:, k, m * MB:(m + 1) * MB],
                            rhs=wsb[:, k, :],
                            start=(k == 0),
                            stop=(k == KCH - 1),
                        )
                    if m == 0:
                        mm.then_inc(sem_mm, 1)
                    else:
                        nc.tensor.drain().then_inc(sem_mm, 1)
        # DVE: copy each half as soon as its accumulation closes; per-copy
        # drain so each store reads settled SBUF. (A then_inc-gated store
        # without the drain benched identically, so the drain is free
        # insurance here.)
        for m in range(NMB):
            nc.vector.wait_ge(sem_mm, m + 1)
            nc.vector.tensor_copy(osb[:, m, :], pts[m][:])
            nc.vector.drain().then_inc(sem_cp, 1)
        # SP: per-half stores
        for m in range(NMB):
            nc.sync.wait_ge(sem_cp, m + 1)
            nc.sync.dma_start(
                out=out[m * MB:(m + 1) * MB, :], in_=osb[:, m, :]
            ).then_inc(sem_out, 16)

        _hoist_first(nc, in_dmas)
    nc.compile()
    return nc


def _prep_inputs(x, W):
    """Per-core host-side layout: SBUF-native [128, KCH, *] f16 arrays."""
    xs = np.ascontiguousarray(x[..., 0], dtype=np.float32)      # [B, P, 8]
    W = np.asarray(W, dtype=np.float32)
    in_maps = []
    for c in range(NCORES):
        pr = slice(c * PL, (c + 1) * PL)
        # x^T chunk: [128, KCH, B] with k_local = kc*128 + kp = p_local*8 + j
        xl = xs[:, pr, :].reshape(B, KL).T                      # [KL, B]
        xl = xl.reshape(KCH, 128, B).transpose(1, 0, 2)         # [128, KCH, B]
        # W2 chunk: W2[(p_local, j), (d, i)] = W[p, d, i, j]
        wl = W[pr].transpose(0, 3, 1, 2).reshape(KL, N_OUT)     # [KL, 160]
        wl = wl.reshape(KCH, 128, N_OUT).transpose(1, 0, 2)     # [128, KCH, 160]
        in_maps.append({
            "xt": np.ascontiguousarray(xl, dtype=np.float16),
            "wt": np.ascontiguousarray(wl, dtype=np.float16),
        })
    return in_maps


def _squash(S):
    """S: [B, 160] summed partials -> squash over each group of 16."""
    S = S.reshape(B, D, VD)
    sq = np.sum(S * S, axis=2, keepdims=True)
    v = S * sq / (1.0 + sq) / np.sqrt(sq + 1e-9)
    return v[..., None].astype(np.float32)                      # [B, D, 16, 1]


def run(x, W, trace=False):
    if "nc" not in _cache:
        _cache["nc"] = _build()
    nc = _cache["nc"]
    in_maps = _prep_inputs(x, W)
    try:
        res = run_bass_kernel_spmd(nc, in_maps, core_ids=list(range(NCORES)), trace=trace)
    except Exception:
        # one retry absorbs transient runtime hiccups
        res = run_bass_kernel_spmd(nc, in_maps, core_ids=list(range(NCORES)), trace=trace)
    S = np.zeros((B, N_OUT), dtype=np.float32)
    for c in range(NCORES):
        S += res.results[c]["out"]
    return _squash(S), res


def kernel(x, W):
    out, _ = run(np.asarray(x), np.asarray(W))
    return out


# revision 40
# speedup vs baseline: 1.7699x; 1.0013x over previous
"""Trainium2 Bass kernel for nn_DigitLayer (CapsNet digit-capsule layer).

Math note: the reference's routing softmax acts on a size-1 axis, so the
coupling coefficients are exactly 1.0 on every iteration and the whole
3-iteration routing loop collapses to

    S[b,d,i] = sum_{p,j} W[p,d,i,j] * x[b,p,j]
    out      = squash(S)  over i (the 16-dim)

i.e. one [B, P*8] @ [P*8, D*16] matmul plus a per-(b,d) squash.

Distribution: the contraction dim P (1152) is sharded across the 8 cores so
every byte of x and W is read from HBM exactly once chip-wide (pure data
parallel would re-read the replicated 5.9MB W on every core: ~7.1MB/core of
HBM traffic vs ~2.0MB/core here). Each core computes a partial
S[b, (d,i)] = sum_k xT[k,b] * W2[k,(d,i)] over its P-shard for all 256
batches via 18 accumulating PE matmuls; the host sums the 8 partial tensors
and applies the (collapsed-routing) squash.

Inputs are fed to the device as float16: the PE runs f16 at full rate
(1 cycle/row vs 4 for fp32) and the DMA bytes halve; measured end-to-end
relative error is ~4e-4 (f16 keeps 11 mantissa bits and accumulation is
fp32 in PSUM).

Device-side layout (per core, all host-prepped, SBUF-native):
    xt [128, 9, 256] f16 : xT chunks, k_local = kc*128 + kp = p_local*8 + j
    wt [128, 9, 160] f16 : W2 chunks, same k mapping, n = d*16 + i
    out [256, 160] f32   : partial S
"""

import numpy as np

import concourse.bacc as bacc
import concourse.mybir as mybir
from concourse.bass_utils import run_bass_kernel_spmd

B, P, D, VP, VD = 256, 1152, 10, 8, 16
NCORES = 8
PL = P // NCORES           # 144 primary capsules per core
KL = PL * VP               # 1152 local contraction length
KCH = KL // 128            # 9 k-chunks of 128
N_OUT = D * VD             # 160
MB = 128                   # batch chunk (matmul M / PSUM partitions)
NMB = B // MB              # 2
GROUPS = (1, 4, 4)         # k-chunks per input-DMA group

_cache = {}


def _hoist_first(nc, instrs):
    """Move the given instructions to the front of their engine's stream so
    the input DMAs issue before the framework preamble (const memsets +
    all-engine barrier) and their transfer latency overlaps it."""
    names = {i.name for i in instrs}
    for bb in nc.main_func.blocks:
        if not any(ins.name in names for ins in bb.instructions):
            continue
        by_engine = {}
        for ins in bb.instructions:
            if ins.name in names:
                by_engine.setdefault(ins.engine, []).append(ins)
        new = []
        emitted = set()
        for ins in bb.instructions:
            if ins.name in names:
                continue
            e = ins.engine
            if e in by_engine and e not in emitted:
                new.extend(by_engine[e])
                emitted.add(e)
            new.append(ins)
        for e, lst in by_engine.items():
            if e not in emitted:
                new.extend(lst)
        bb.instructions[:] = new


def _build():
    """Raw-bass kernel (no TileContext), hand-placed semaphores.

    Hard-won rules baked in here:
      * One semaphore per DMA: a HWDGE DMA completes as 16 unordered +1
        sub-increments, so intermediate thresholds on a shared sem race.
      * The PE gate must wait on the DMA completion semaphores; an engine
        DRAIN does NOT barrier HWDGE DMA data (cold-run NaNs).
      * PE drain before the DVE reads PSUM: the stop-matmul retire does not
        guarantee the PSUM writes drained, and DVE reading a bank the PE is
        still writing is fatal on HW. Same for DVE drain before the store.
      * No wait on the output DMA semaphore: the walrus end-of-program
        barrier covers it (verified bitwise against the waiting variant).
    """
    dt_in = mybir.dt.float16
    NG = len(GROUPS)
    starts = [sum(GROUPS[:i]) for i in range(NG)]
    nc = bacc.Bacc("TRN2", debug=False, num_devices=NCORES)
    xt = nc.dram_tensor("xt", [128, KCH, B], dt_in, kind="ExternalInput").ap()
    wt = nc.dram_tensor("wt", [128, KCH, N_OUT], dt_in, kind="ExternalInput").ap()
    out = nc.dram_tensor("out", [B, N_OUT], mybir.dt.float32, kind="ExternalOutput").ap()

    from contextlib import ExitStack
    with ExitStack() as ctx:
        xsb = ctx.enter_context(nc.sbuf_tensor([128, KCH, B], dt_in))
        wsb = ctx.enter_context(nc.sbuf_tensor([128, KCH, N_OUT], dt_in))
        osb = ctx.enter_context(nc.sbuf_tensor([MB, NMB, N_OUT], mybir.dt.float32))
        pts = [
            ctx.enter_context(nc.psum_tensor(f"pt{m}", [MB, N_OUT], mybir.dt.float32))
            for m in range(NMB)
        ]
        sem_xs = [ctx.enter_context(nc.semaphore(name=f"sem_x{g}")) for g in range(NG)]
        sem_ws = [ctx.enter_context(nc.semaphore(name=f"sem_w{g}")) for g in range(NG)]
        sem_mm = ctx.enter_context(nc.semaphore(name="sem_mm"))
        sem_cp = ctx.enter_context(nc.semaphore(name="sem_cp"))
        sem_out = ctx.enter_context(nc.semaphore(name="sem_out"))

        # input DMAs: x on the SP HWDGE ring, w on the ACT ring (parallel)
        in_dmas = []
        for g in range(NG):
            k0, kn = starts[g], GROUPS[g]
            in_dmas.append(nc.sync.dma_start(
                out=xsb[:, k0:k0 + kn, :], in_=xt[:, k0:k0 + kn, :]
            ).then_inc(sem_xs[g], 16).ins)
        for g in range(NG):
            k0, kn = starts[g], GROUPS[g]
            in_dmas.append(nc.scalar.dma_start(
                out=wsb[:, k0:k0 + kn, :], in_=wt[:, k0:k0 + kn, :]
            ).then_inc(sem_ws[g], 16).ins)

        # PE: 18 accumulating matmuls, m INNER (the two batch halves
        # accumulate into separate PSUM banks concurrently), group-gated on
        # the input DMAs — so after the last DMA gate only the last group's
        # matmuls remain, not a whole batch half. In the last group, half 0
        # closes first so its copy/store overlap half 1's matmuls (the copy
        # reads a different PSUM bank than the PE is writing).
        for g in range(NG):
            k0, kn = starts[g], GROUPS[g]
            nc.tensor.wait_ge(sem_xs[g], 16)
            nc.tensor.wait_ge(sem_ws[g], 16)
            if g < NG - 1:
                for kk in range(kn):
                    k = k0 + kk
                    for m in range(NMB):
                        nc.tensor.matmul(
                            pts[m][:],
                            lhsT=xsb[:, k, m * MB:(m + 1) * MB],
                            rhs=wsb[:, k, :],
                            start=(k == 0),
                            stop=(k == KCH - 1),
                        )
            else:
                for m in range(NMB):
                    for kk in range(kn):
                        k = k0 + kk
                        mm = nc.tensor.matmul(
                            pts[m][:],
                            lhsT=xsb[:, k, m * MB:(m + 1) * MB],
                            rhs=wsb[:, k, :],
                            start=(k == 0),
                            stop=(k == KCH - 1),
                        )
                    if m == 0:
                        mm.then_inc(sem_mm, 1)
                    else:
                        nc.tensor.drain().then_inc(sem_mm, 1)
        # DVE: copy each half as soon as its accumulation closes; per-copy
        # drain so each store reads settled SBUF. (A then_inc-gated store
        # without the drain benched identically, so the drain is free
        # insurance here.)
        for m in range(NMB):
            nc.vector.wait_ge(sem_mm, m + 1)
            nc.vector.tensor_copy(osb[:, m, :], pts[m][:])
            nc.vector.drain().then_inc(sem_cp, 1)
        # SP: per-half stores
        for m in range(NMB):
            nc.sync.wait_ge(sem_cp, m + 1)
            nc.sync.dma_start(
                out=out[m * MB:(m + 1) * MB, :], in_=osb[:, m, :]
            ).then_inc(sem_out, 16)

        _hoist_first(nc, in_dmas)
    nc.compile()
    return nc


def _prep_inputs(x, W):
    """Per-core host-side layout: SBUF-native [128, KCH, *] f16 arrays."""
    xs = np.ascontiguousarray(x[..., 0], dtype=np.float32)      # [B, P, 8]
    W = np.asarray(W, dtype=np.float32)
    in_maps = []
    for c in range(NCORES):
        pr = slice(c * PL, (c + 1) * PL)
        # x^T chunk: [128, KCH, B] with k_local = kc*128 + kp = p_local*8 + j
        xl = xs[:, pr, :].reshape(B, KL).T                      # [KL, B]
        xl = xl.reshape(KCH, 128, B).transpose(1, 0, 2)         # [128, KCH, B]
        # W2 chunk: W2[(p_local, j), (d, i)] = W[p, d, i, j]
        wl = W[pr].transpose(0, 3, 1, 2).reshape(KL, N_OUT)     # [KL, 160]
        wl = wl.reshape(KCH, 128, N_OUT).transpose(1, 0, 2)     # [128, KCH, 160]
        in_maps.append({
            "xt": np.ascontiguousarray(xl, dtype=np.float16),
            "wt": np.ascontiguousarray(wl, dtype=np.float16),
        })
    return in_maps


def _squash(S):
    """S: [B, 160] summed partials -> squash over each group of 16."""
    S = S.reshape(B, D, VD)
    sq = np.sum(S * S, axis=2, keepdims=True)
    v = S * sq / (1.0 + sq) / np.sqrt(sq + 1e-9)
    return v[..., None].astype(np.float32)                      # [B, D, 16, 1]


def run(x, W, trace=False):
    if "nc" not in _cache:
        _cache["nc"] = _build()
    nc = _cache["nc"]
    in_maps = _prep_inputs(x, W)
    try:
        res = run_bass_kernel_spmd(nc, in_maps, core_ids=list(range(NCORES)), trace=trace)
    except Exception:
        # one retry absorbs transient runtime hiccups
        res = run_bass_kernel_spmd(nc, in_maps, core_ids=list(range(NCORES)), trace=trace)
    S = np.zeros((B, N_OUT), dtype=np.float32)
    for c in range(NCORES):
        S += res.results[c]["out"]
    return _squash(S), res


def kernel(x, W):
    out, _ = run(np.asarray(x), np.asarray(W))
    return out


# revision 42
# speedup vs baseline: 1.7735x; 1.0020x over previous
"""Trainium2 Bass kernel for nn_DigitLayer (CapsNet digit-capsule layer).

Math note: the reference's routing softmax acts on a size-1 axis, so the
coupling coefficients are exactly 1.0 on every iteration and the whole
3-iteration routing loop collapses to

    S[b,d,i] = sum_{p,j} W[p,d,i,j] * x[b,p,j]
    out      = squash(S)  over i (the 16-dim)

i.e. one [B, P*8] @ [P*8, D*16] matmul plus a per-(b,d) squash.

Distribution: the contraction dim P (1152) is sharded across the 8 cores so
every byte of x and W is read from HBM exactly once chip-wide (pure data
parallel would re-read the replicated 5.9MB W on every core: ~7.1MB/core of
HBM traffic vs ~2.0MB/core here). Each core computes a partial
S[b, (d,i)] = sum_k xT[k,b] * W2[k,(d,i)] over its P-shard for all 256
batches via 18 accumulating PE matmuls; the host sums the 8 partial tensors
and applies the (collapsed-routing) squash.

Inputs are fed to the device as float16: the PE runs f16 at full rate
(1 cycle/row vs 4 for fp32) and the DMA bytes halve; measured end-to-end
relative error is ~4e-4 (f16 keeps 11 mantissa bits and accumulation is
fp32 in PSUM).

Device-side layout (per core, all host-prepped, SBUF-native):
    xt [128, 9, 256] f16 : xT chunks, k_local = kc*128 + kp = p_local*8 + j
    wt [128, 9, 160] f16 : W2 chunks, same k mapping, n = d*16 + i
    out [256, 160] f32   : partial S
"""

import numpy as np

import concourse.bacc as bacc
import concourse.mybir as mybir
from concourse.bass_utils import run_bass_kernel_spmd

B, P, D, VP, VD = 256, 1152, 10, 8, 16
NCORES = 8
PL = P // NCORES           # 144 primary capsules per core
KL = PL * VP               # 1152 local contraction length
KCH = KL // 128            # 9 k-chunks of 128
N_OUT = D * VD             # 160
MB = 128                   # batch chunk (matmul M / PSUM partitions)
NMB = B // MB              # 2
GROUPS = (1, 4, 4)         # k-chunks per input-DMA group

_cache = {}


def _hoist_first(nc, instrs):
    """Move the given instructions to the front of their engine's stream so
    the input DMAs issue before the framework preamble (const memsets +
    all-engine barrier) and their transfer latency overlaps it."""
    names = {i.name for i in instrs}
    for bb in nc.main_func.blocks:
        if not any(ins.name in names for ins in bb.instructions):
            continue
        by_engine = {}
        for ins in bb.instructions:
            if ins.name in names:
                by_engine.setdefault(ins.engine, []).append(ins)
        new = []
        emitted = set()
        for ins in bb.instructions:
            if ins.name in names:
                continue
            e = ins.engine
            if e in by_engine and e not in emitted:
                new.extend(by_engine[e])
                emitted.add(e)
            new.append(ins)
        for e, lst in by_engine.items():
            if e not in emitted:
                new.extend(lst)
        bb.instructions[:] = new


def _build():
    """Raw-bass kernel (no TileContext), hand-placed semaphores.

    Hard-won rules baked in here:
      * One semaphore per DMA: a HWDGE DMA completes as 16 unordered +1
        sub-increments, so intermediate thresholds on a shared sem race.
      * The PE gate must wait on the DMA completion semaphores; an engine
        DRAIN does NOT barrier HWDGE DMA data (cold-run NaNs).
      * PE drain before the DVE reads PSUM: the stop-matmul retire does not
        guarantee the PSUM writes drained, and DVE reading a bank the PE is
        still writing is fatal on HW. Same for DVE drain before the store.
      * No wait on the output DMA semaphore: the walrus end-of-program
        barrier covers it (verified bitwise against the waiting variant).
    """
    dt_in = mybir.dt.float16
    NG = len(GROUPS)
    starts = [sum(GROUPS[:i]) for i in range(NG)]
    nc = bacc.Bacc("TRN2", debug=False, num_devices=NCORES)
    xt = nc.dram_tensor("xt", [128, KCH, B], dt_in, kind="ExternalInput").ap()
    wt = nc.dram_tensor("wt", [128, KCH, N_OUT], dt_in, kind="ExternalInput").ap()
    out = nc.dram_tensor("out", [B, N_OUT], mybir.dt.float32, kind="ExternalOutput").ap()

    from contextlib import ExitStack
    with ExitStack() as ctx:
        xsb = ctx.enter_context(nc.sbuf_tensor([128, KCH, B], dt_in))
        wsb = ctx.enter_context(nc.sbuf_tensor([128, KCH, N_OUT], dt_in))
        osb = ctx.enter_context(nc.sbuf_tensor([MB, NMB, N_OUT], mybir.dt.float32))
        pts = [
            ctx.enter_context(nc.psum_tensor(f"pt{m}", [MB, N_OUT], mybir.dt.float32))
            for m in range(NMB)
        ]
        sem_xs = [ctx.enter_context(nc.semaphore(name=f"sem_x{g}")) for g in range(NG)]
        sem_ws = [ctx.enter_context(nc.semaphore(name=f"sem_w{g}")) for g in range(NG)]
        sem_mm = ctx.enter_context(nc.semaphore(name="sem_mm"))
        sem_cp = ctx.enter_context(nc.semaphore(name="sem_cp"))
        sem_out = ctx.enter_context(nc.semaphore(name="sem_out"))

        # input DMAs: x on the SP HWDGE ring, w on the ACT ring (parallel)
        in_dmas = []
        for g in range(NG):
            k0, kn = starts[g], GROUPS[g]
            in_dmas.append(nc.sync.dma_start(
                out=xsb[:, k0:k0 + kn, :], in_=xt[:, k0:k0 + kn, :]
            ).then_inc(sem_xs[g], 16).ins)
        for g in range(NG):
            k0, kn = starts[g], GROUPS[g]
            in_dmas.append(nc.scalar.dma_start(
                out=wsb[:, k0:k0 + kn, :], in_=wt[:, k0:k0 + kn, :]
            ).then_inc(sem_ws[g], 16).ins)

        # PE: 18 accumulating matmuls, m INNER (the two batch halves
        # accumulate into separate PSUM banks concurrently), group-gated on
        # the input DMAs — so after the last DMA gate only the last group's
        # matmuls remain, not a whole batch half. In the last group, half 0
        # closes first so its copy/store overlap half 1's matmuls (the copy
        # reads a different PSUM bank than the PE is writing).
        for g in range(NG):
            k0, kn = starts[g], GROUPS[g]
            nc.tensor.wait_ge(sem_xs[g], 16)
            nc.tensor.wait_ge(sem_ws[g], 16)
            if g < NG - 1:
                for kk in range(kn):
                    k = k0 + kk
                    for m in range(NMB):
                        nc.tensor.matmul(
                            pts[m][:],
                            lhsT=xsb[:, k, m * MB:(m + 1) * MB],
                            rhs=wsb[:, k, :],
                            start=(k == 0),
                            stop=(k == KCH - 1),
                        )
            else:
                for m in range(NMB):
                    for kk in range(kn):
                        k = k0 + kk
                        mm = nc.tensor.matmul(
                            pts[m][:],
                            lhsT=xsb[:, k, m * MB:(m + 1) * MB],
                            rhs=wsb[:, k, :],
                            start=(k == 0),
                            stop=(k == KCH - 1),
                        )
                    if m == 0:
                        mm.then_inc(sem_mm, 1)
                    else:
                        nc.tensor.drain().then_inc(sem_mm, 1)
        # DVE: copy each half as soon as its accumulation closes; per-copy
        # drain so each store reads settled SBUF. (A then_inc-gated store
        # without the drain benched identically, so the drain is free
        # insurance here.)
        for m in range(NMB):
            nc.vector.wait_ge(sem_mm, m + 1)
            nc.vector.tensor_copy(osb[:, m, :], pts[m][:])
            nc.vector.drain().then_inc(sem_cp, 1)
        # SP: per-half stores
        for m in range(NMB):
            nc.sync.wait_ge(sem_cp, m + 1)
            nc.sync.dma_start(
                out=out[m * MB:(m + 1) * MB, :], in_=osb[:, m, :]
            ).then_inc(sem_out, 16)

        _hoist_first(nc, in_dmas)
    nc.compile()
    return nc


def _prep_inputs(x, W):
    """Per-core host-side layout: SBUF-native [128, KCH, *] f16 arrays."""
    xs = np.ascontiguousarray(x[..., 0], dtype=np.float32)      # [B, P, 8]
    W = np.asarray(W, dtype=np.float32)
    in_maps = []
    for c in range(NCORES):
        pr = slice(c * PL, (c + 1) * PL)
        # x^T chunk: [128, KCH, B] with k_local = kc*128 + kp = p_local*8 + j
        xl = xs[:, pr, :].reshape(B, KL).T                      # [KL, B]
        xl = xl.reshape(KCH, 128, B).transpose(1, 0, 2)         # [128, KCH, B]
        # W2 chunk: W2[(p_local, j), (d, i)] = W[p, d, i, j]
        wl = W[pr].transpose(0, 3, 1, 2).reshape(KL, N_OUT)     # [KL, 160]
        wl = wl.reshape(KCH, 128, N_OUT).transpose(1, 0, 2)     # [128, KCH, 160]
        in_maps.append({
            "xt": np.ascontiguousarray(xl, dtype=np.float16),
            "wt": np.ascontiguousarray(wl, dtype=np.float16),
        })
    return in_maps


def _squash(S):
    """S: [B, 160] summed partials -> squash over each group of 16."""
    S = S.reshape(B, D, VD)
    sq = np.sum(S * S, axis=2, keepdims=True)
    v = S * sq / (1.0 + sq) / np.sqrt(sq + 1e-9)
    return v[..., None].astype(np.float32)                      # [B, D, 16, 1]


def run(x, W, trace=False):
    if "nc" not in _cache:
        _cache["nc"] = _build()
    nc = _cache["nc"]
    in_maps = _prep_inputs(x, W)
    try:
        res = run_bass_kernel_spmd(nc, in_maps, core_ids=list(range(NCORES)), trace=trace)
    except Exception:
        # one retry absorbs transient runtime hiccups
        res = run_bass_kernel_spmd(nc, in_maps, core_ids=list(range(NCORES)), trace=trace)
    S = np.zeros((B, N_OUT), dtype=np.float32)
    for c in range(NCORES):
        S += res.results[c]["out"]
    return _squash(S), res


def kernel(x, W):
    out, _ = run(np.asarray(x), np.asarray(W))
    return out
